# revision 1
# baseline (speedup 1.0000x reference)
"""Trainium2 Bass kernel for AnatomicalMaskedLinear (block-masked dense layer).

Reference op:
    mask  = kron(adjacency, ones(256, 128))            # (21*256, 21*128)
    y     = x.reshape(B, 21*128) @ (weight*mask).T + bias
    out   = y.reshape(B, 21, 256)

Strategy (v2):
  * Only nonzero (256o x 128i) blocks are shipped/matmul'd. 8 cores =
    4 batch quarters x 2 node-row halves; all cores run one SPMD graph.
  * The stream mixes fp16 with fp8e4m3 DoubleRow: ~25% of the slots are
    covered by 3 global disjoint j-pairs, each pair computed as one
    DoubleRow matmul (2 k-blocks per MM at ~2x rate). Scales split as
    W*16 / x/16 so fp8 products share the fp16 PSUM scale in one
    accumulation group. Rel err 0.0161 vs the 2e-2 gate (deterministic:
    fixed inputs, fixed schedule). Everything else keeps the PE gap-free
    as early as possible:
      - whole W (fp16, slot-packed) and x (fp16, phase-major, first-use
        order) live in SBUF; prefix DMAs are demand-ordered ~0.25-0.4MB
        chunks balanced across the two HWDGE queues (sync/scalar) — small
        enough to pipeline through the ~8 shared in-flight DMA semaphore
        lanes, big enough to amortize descriptor-gen. Stores ride gpsimd
        (SWDGE) during the load-critical window so no load ever queues
        behind a compute-dependent store. (SWDGE loads and >0.5MB jumbos
        were both measured slower.)
      - batch is processed in 2 phases of 512 cols so only half of x
        gates the stream prefix.
      - node order is hill-climbed against a measured two-rate DMA
        delivery model (data starts ~3.5us into the window, ~330 GB/s
        ramping to ~400 GB/s).
      - 8 garbage warm-up matmuls ramp the PE clock (HAM) toward 8/8
        while the first DMAs are in flight; more would delay the stream
        1:1, fewer leave the first real matmuls at the cold 427ns rate.
      - the last node's phase-1 work runs as two 256-col accumulation
        groups: the first half's bias-add/store hides under the second
        half's matmuls, and the two stores land on the two idle HWDGE
        queues.
  * Output is stored fp16 (halves the 11MB store traffic; quantization
    adds ~2e-4 error against a 2e-2 budget) and upconverted on host.
  * Measured: 115.1us HW exec (fp16-only variant: 123.6-124.0; baseline
    125.8). The device intermittently throttles to ~2.0GHz after
    sustained load (identical code then measures ~145-149us). The
    remaining gap to the ~102.4us stream floor is framework overhead
    (~1.7us preamble-to-first-tensor-op, ~3us DMA issue/HBM latency
    before first data, ~4-6us early-stream DMA stalls while the 10.3MB
    phase-0 prefix lands, ~3us drain, ~8us walrus end-of-iteration
    semaphore reset that is counted inside the measured window).
"""

import os
import numpy as np

NUM_NODES = 21
IN_F = 128
OUT_F = 256
BATCH = 4096
N_CORES = 8
P_BATCH = 4                      # batch ways
B_C = BATCH // P_BATCH           # 1024 batch rows per core
B_TILE = 512                     # matmul moving free dim (one phase)
N_PH = 2                         # batch phases per core
K_TOTAL = NUM_NODES * IN_F       # 2688
O_C = NUM_NODES * 128            # 2688 out rows per core (half of each node)

_CACHE = {}                      # schedule key -> (nc, sched, xorder)

# analytic model constants for the node-order optimizer
_MM_NS = 216.0                   # per 512-col fp16 matmul, warm
_ISSUE = 3400.0                  # ns (in-window) before DMA data flows


def _t_ready(nbytes):
    """Two-rate delivery model fitted from traces (ns after window start)."""
    slow_rate, slow_win, fast_rate = 330.0, 12000.0, 400.0
    slow_cap = slow_rate * slow_win
    if nbytes <= slow_cap:
        return _ISSUE + nbytes / slow_rate
    return _ISSUE + slow_win + (nbytes - slow_cap) / fast_rate


def _stall_bound(order, active, pair_js=frozenset()):
    """Worst (data-ready - mm-schedule) over phase-0/1 checkpoints."""
    xseen = set()
    xb, wb, mmper = [], [], []
    cx = cw = 0
    for i in order:
        js = active[i]
        new = [j for j in js if j not in xseen]
        xseen |= set(new)
        cx += len(new) * 128 * B_TILE * 2
        cw += max(len(js), 1) * 128 * 128 * 2
        xb.append(cx)
        wb.append(cw)
        # DoubleRow-covered slots run ~2 blocks per ~0.65 fp16-MM time
        n8 = len([j for j in js if j in pair_js])
        n16 = max(len(js), 1) - n8
        mmper.append((n16 + 0.65 * n8) * _MM_NS)
    worst = -1e18
    cm = 5200.0          # warm-up matmuls run until ~5.2us in-window
    for p in range(N_PH):
        for k in range(len(order)):
            need = xb[-1] * p + xb[k] + (wb[k] if p == 0 else wb[-1])
            stall = _t_ready(need) - cm
            if stall > worst:
                worst = stall
            cm += mmper[k]
    return worst


def _node_order(active):
    """Greedy seed + deterministic hill-climb on the DMA stall bound."""
    import random
    # approximate DR coverage for the mm-time model: js that will end up in
    # global pairs run faster; _choose_pairs is order-independent, so a
    # rough pre-pass here keeps the model honest.
    A = np.zeros((NUM_NODES, NUM_NODES), dtype=bool)
    for i, js in active.items():
        if js:
            A[i, list(js)] = True
    tmp_sched = tuple((i, tuple(js) if js else (0,), not js)
                      for i, js in active.items())
    try:
        pairs, _cov = _choose_pairs(None, tmp_sched)
        pair_js = frozenset(j for p in pairs for j in p)
    except Exception:
        pair_js = frozenset()
    loaded = set()
    remaining = set(range(NUM_NODES))
    order = []
    while remaining:
        nxt = min(remaining,
                  key=lambda i: (len(set(active[i]) - loaded),
                                 len(active[i]), i))
        order.append(nxt)
        loaded |= set(active[nxt])
        remaining.remove(nxt)
    rnd = random.Random(0)
    cur = list(order)
    curs = _stall_bound(cur, active, pair_js)
    n = len(cur)
    for _ in range(8000):
        a, b = rnd.sample(range(n), 2)
        cur[a], cur[b] = cur[b], cur[a]
        s = _stall_bound(cur, active, pair_js)
        if s <= curs:
            curs = s
        else:
            cur[a], cur[b] = cur[b], cur[a]
    return cur


def _build_schedule(adjacency):
    """[(i, [j...], zero_pad)] in optimized node order; >=1 slot per node."""
    A = np.asarray(adjacency) != 0
    active = {i: [int(j) for j in np.where(A[i])[0]] for i in range(NUM_NODES)}
    sched = []
    for i in _node_order(active):
        js = active[i]
        if js:
            sched.append((i, tuple(js), False))
        else:
            sched.append((i, (0,), True))
    return tuple(sched)


def _choose_pairs(adjacency, sched, frac=0.22):
    """Greedy disjoint global j-pairs for fp8-DoubleRow coverage.

    Returns (pairs, cov) where pairs = [(jA, jB), ...] and cov[k] = list of
    (pair_idx, jA, jB) covered at sched position k. A covered pair replaces
    two fp16 slots of that node with one DoubleRow matmul.
    """
    A = np.zeros((NUM_NODES, NUM_NODES), dtype=bool)
    for i, js, zero in sched:
        if not zero:
            A[i, list(js)] = True
    S = sum(len(js) for _, js, _ in sched)
    used = set()
    pairs = []
    covered = 0
    while covered < frac * S:
        best = None
        for a in range(NUM_NODES):
            if a in used:
                continue
            for b in range(a + 1, NUM_NODES):
                if b in used:
                    continue
                n = int(np.sum(A[:, a] & A[:, b]))
                if best is None or n > best[0]:
                    best = (n, a, b)
        if best is None or best[0] == 0:
            break
        n, a, b = best
        used |= {a, b}
        pairs.append((a, b))
        covered += 2 * n
    cov = []
    for _k, (i, js, zero) in enumerate(sched):
        jset = set(js)
        pc = []
        if not zero:
            for pi, (a, b) in enumerate(pairs):
                if a in jset and b in jset:
                    pc.append((pi, a, b))
        cov.append(pc)
    return pairs, cov


def _x_first_use(sched):
    """x blocks in first-use order (only blocks actually used)."""
    xorder = []
    seen = set()
    for _i, js, _z in sched:
        for j in js:
            if j not in seen:
                seen.add(j)
                xorder.append(j)
    return xorder


def _build_graph(sched):
    import concourse.tile as tile
    from concourse import bacc, mybir

    xorder = _x_first_use(sched)
    xpos = {j: s for s, j in enumerate(xorder)}
    NX = len(xorder)
    f32 = mybir.dt.float32
    f16 = mybir.dt.float16
    f8 = mybir.dt.float8e4

    pairs, cov = _choose_pairs(None, sched)
    NPAIR = len(pairs)
    # per-node fp16 slots after removing DoubleRow-covered js
    js16_all = []
    droff = []
    ncov = 0
    for k, (i, js, zero) in enumerate(sched):
        cj = {j for _pi, a, b in cov[k] for j in (a, b)}
        js16_all.append(tuple(j for j in js if j not in cj))
        droff.append(ncov)
        ncov += len(cov[k])
    NCOV = ncov
    S16 = sum(len(js16) for js16 in js16_all)

    nc = bacc.Bacc("TRN2", target_bir_lowering=False, debug=False,
                   num_devices=N_CORES)

    xt_d = nc.declare_dram_parameter("xt", [128, N_PH * NX * B_TILE], f16,
                                     isOutput=False)
    wp_d = nc.declare_dram_parameter("wp", [128, max(S16, 1) * 128], f16,
                                     isOutput=False)
    wq8_d = nc.declare_dram_parameter("wq8", [128, 2, max(NCOV, 1) * 128],
                                      f8, isOutput=False)
    xq8_d = nc.declare_dram_parameter("xq8",
                                      [128, 2, max(NPAIR, 1) * N_PH * B_TILE],
                                      f8, isOutput=False)
    bias_d = nc.declare_dram_parameter("biasr", [128, NUM_NODES], f32,
                                       isOutput=False)
    # fp16 output stores halve the 11MB store traffic (error ~5e-4, far
    # inside the 2e-2 gate); the host upconverts to f32.
    out_d = nc.declare_dram_parameter("out", [O_C, B_C], f16, isOutput=True)

    # ---- DMA plan: demand-ordered prefix in ~0.25-0.5MB chunks balanced
    # across the two HWDGE queues (HWDGE has ~8 shared in-flight semaphore
    # lanes, so many medium DMAs pipeline; big jumbos serialize issue).
    # A few mid-prefix w chunks ride the otherwise-idle SWDGE queue.
    GPSIMD_W_NODES = ()           # SWDGE loads measured slower than HWDGE
    items = []          # ("w"|"x0", a, b) -> slot or x-s ranges
    gp_items = []
    slot0 = []
    s = 0
    xdone = 0
    seen = set()
    x8seen = set()
    for k, (i, js, _z) in enumerate(sched):
        js16 = js16_all[k]
        slot0.append(s)
        if js16:
            if k == 0 and len(js16) >= 4:
                mid = s + len(js16) // 2
                items.append(("w", s, mid))
                items.append(("w", mid, s + len(js16)))
            else:
                items.append(("w", s, s + len(js16)))
        if cov[k]:
            items.append(("w8", droff[k], droff[k] + len(cov[k])))
            for pi, _a, _b in cov[k]:
                if pi not in x8seen:
                    x8seen.add(pi)
                    items.append(("x8", pi, 0))
        s += len(js16)
        new = [j for j in js if j not in seen]
        seen |= set(new)
        end = xdone + len(new)
        csz = 1 if xdone == 0 else (2 if k == 0 else 3)
        while xdone < end:
            e = min(xdone + csz, end)
            items.append(("x0", xdone, e))
            xdone = e
            csz = 2 if k == 0 else 3
    qb = [0, 0]
    qitems = [[], []]
    for it in items:
        kind, a, b = it
        if kind == "w":
            nbytes = (b - a) * 128 * 128 * 2
        elif kind == "x0":
            nbytes = (b - a) * 128 * B_TILE * 2
        elif kind == "w8":
            nbytes = (b - a) * 128 * 256
        else:
            nbytes = 128 * 2 * B_TILE
        qi = 0 if qb[0] <= qb[1] else 1
        qitems[qi].append(it)
        qb[qi] += nbytes

    last_k = len(sched) - 1

    with tile.TileContext(nc) as tc:
        with (
            tc.tile_pool(name="persist", bufs=1) as persist,
            tc.tile_pool(name="psum", bufs=6, space="PSUM") as psump,
            tc.tile_pool(name="psumh", bufs=2, space="PSUM") as psumh,
            tc.tile_pool(name="outp", bufs=8) as outp,
            tc.tile_pool(name="tailp", bufs=1) as tailp,
        ):
            warm = persist.tile([128, B_TILE], f16, tag="warm")
            nc.gpsimd.memset(warm[:], 0.0)
            bias_sb = persist.tile([128, NUM_NODES], f32, tag="bias")
            nc.gpsimd.dma_start(out=bias_sb[:], in_=bias_d[:])

            xt = persist.tile([128, N_PH * NX * B_TILE], f16, tag="xt")
            w_all = persist.tile([128, max(S16, 1) * 128], f16, tag="wall")
            wq8 = persist.tile([128, 2, max(NCOV, 1) * 128], f8, tag="wq8")
            xq8 = persist.tile([128, 2, max(NPAIR, 1) * N_PH * B_TILE], f8,
                               tag="xq8")

            for qi, eng in ((0, nc.sync), (1, nc.scalar)):
                for kind, a, b in qitems[qi]:
                    if kind == "w":
                        eng.dma_start(out=w_all[:, a * 128:b * 128],
                                      in_=wp_d[:, a * 128:b * 128])
                    elif kind == "x0":
                        eng.dma_start(out=xt[:, a * B_TILE:b * B_TILE],
                                      in_=xt_d[:, a * B_TILE:b * B_TILE])
                    elif kind == "w8":
                        eng.dma_start(out=wq8[:, :, a * 128:b * 128],
                                      in_=wq8_d[:, :, a * 128:b * 128])
                    else:          # ("x8", pair, phase)
                        c0 = (a * N_PH + b) * B_TILE
                        eng.dma_start(out=xq8[:, :, c0:c0 + B_TILE],
                                      in_=xq8_d[:, :, c0:c0 + B_TILE])
            # phase-1 x (and fp8 x) rides sync behind the prefix
            for a in range(0, NX, 4):
                b = min(a + 4, NX)
                nc.sync.dma_start(
                    out=xt[:, (NX + a) * B_TILE:(NX + b) * B_TILE],
                    in_=xt_d[:, (NX + a) * B_TILE:(NX + b) * B_TILE])
            for pi in range(NPAIR):
                c0 = (pi * N_PH + 1) * B_TILE
                nc.sync.dma_start(out=xq8[:, :, c0:c0 + B_TILE],
                                  in_=xq8_d[:, :, c0:c0 + B_TILE])

            # PE clock warm-up on garbage zeros (stream is DMA-bound early,
            # so these are free; they carry the HAM past its cold window and
            # delay the real stream until DMA delivery has enough slack that
            # no stall is long enough to re-cool the clock)
            for wi in range(11):
                wps = psump.tile([128, B_TILE], f32, tag="acc",
                                 name=f"warm_{wi}")
                nc.tensor.matmul(wps[:], warm[:, :128], warm[:],
                                 start=True, stop=True)

            for h in range(N_PH):
                for k, (i, js, _z) in enumerate(sched):
                    nj = len(js)
                    ob = h * B_TILE
                    js16 = js16_all[k]
                    nops = len(cov[k]) + len(js16)
                    if h == N_PH - 1 and k == last_k:
                        # final node: two 256-col accumulation groups so the
                        # first half's evac/store hides under the second
                        # half's matmuls; stores on the two idle HWDGE qs.
                        for c, (evac_eng, st_eng) in enumerate(
                                ((nc.vector, nc.scalar),
                                 (nc.vector, nc.sync))):
                            ph = psumh.tile([128, 256], f32, tag="acch",
                                            name=f"acch_{c}")
                            op = 0
                            for pp, (pi, _a, _b) in enumerate(cov[k]):
                                wc = (droff[k] + pp) * 128
                                xc = (pi * N_PH + h) * B_TILE + c * 256
                                nc.tensor.matmul(
                                    ph[:],
                                    wq8[:, :, wc:wc + 128],
                                    xq8[:, :, xc:xc + 256],
                                    start=(op == 0), stop=(op == nops - 1),
                                    perf_mode=mybir.MatmulPerfMode.DoubleRow,
                                )
                                op += 1
                            for idx, j in enumerate(js16):
                                st = slot0[k] + idx
                                xc = (h * NX + xpos[j]) * B_TILE + c * 256
                                nc.tensor.matmul(
                                    ph[:],
                                    w_all[:, st * 128:(st + 1) * 128],
                                    xt[:, xc:xc + 256],
                                    start=(op == 0), stop=(op == nops - 1),
                                )
                                op += 1
                            oth = tailp.tile([128, 256], f16,
                                             tag=f"oth{c}")
                            evac_eng.tensor_scalar_add(oth[:], ph[:],
                                                       bias_sb[:, i:i + 1])
                            st_eng.dma_start(
                                out=out_d[i * 128:(i + 1) * 128,
                                          ob + c * 256:ob + (c + 1) * 256],
                                in_=oth[:])
                        continue
                    ps = psump.tile([128, B_TILE], f32, tag="acc",
                                    name=f"acc_{h}_{k}")
                    op = 0
                    for pp, (pi, _a, _b) in enumerate(cov[k]):
                        wc = (droff[k] + pp) * 128
                        xc = (pi * N_PH + h) * B_TILE
                        nc.tensor.matmul(
                            ps[:],
                            wq8[:, :, wc:wc + 128],
                            xq8[:, :, xc:xc + B_TILE],
                            start=(op == 0), stop=(op == nops - 1),
                            perf_mode=mybir.MatmulPerfMode.DoubleRow,
                        )
                        op += 1
                    for idx, j in enumerate(js16):
                        st = slot0[k] + idx
                        xc = (h * NX + xpos[j]) * B_TILE
                        nc.tensor.matmul(
                            ps[:],
                            w_all[:, st * 128:(st + 1) * 128],
                            xt[:, xc:xc + B_TILE],
                            start=(op == 0), stop=(op == nops - 1),
                        )
                        op += 1
                    ot = outp.tile([128, B_TILE], f16, tag="ot")
                    nc.vector.tensor_scalar_add(ot[:], ps[:],
                                                bias_sb[:, i:i + 1])
                    eng = nc.gpsimd if (h == 0 and k < 13) else nc.scalar
                    eng.dma_start(
                        out=out_d[i * 128:(i + 1) * 128, ob:ob + B_TILE],
                        in_=ot[:])

    nc.compile()
    return nc


def _get_graph(adjacency):
    sched = _build_schedule(adjacency)
    if sched not in _CACHE:
        _CACHE[sched] = (_build_graph(sched), sched)
    return _CACHE[sched]


def _pack_inputs(x, weight, bias, sched):
    """Build the 8 per-core input maps (host-side slicing/layout only)."""
    import ml_dtypes
    f8np = ml_dtypes.float8_e4m3
    xorder = _x_first_use(sched)
    NX = len(xorder)
    x = np.asarray(x, dtype=np.float32).reshape(BATCH, NUM_NODES, IN_F)
    x16 = x.astype(np.float16)
    weight32 = np.asarray(weight, dtype=np.float32)
    weight = weight32.astype(np.float16)
    bias = np.asarray(bias, dtype=np.float32)

    pairs, cov = _choose_pairs(None, sched)
    NPAIR = max(len(pairs), 1)

    flat = []  # (i, j, zero) in fp16 slot order (DR-covered js removed)
    for k, (i, js, zero) in enumerate(sched):
        cj = {j for _pi, a, b in cov[k] for j in (a, b)}
        for j in js:
            if j not in cj:
                flat.append((i, j, zero))
    S = max(len(flat), 1)
    if not flat:
        flat = [(sched[0][0], 0, True)]

    w5 = weight.reshape(NUM_NODES, 2, 128, NUM_NODES, IN_F)  # i, h, o, j, k
    w5t = w5.transpose(1, 4, 0, 3, 2)                        # h, k, i, j, o

    si = np.array([f[0] for f in flat])
    sj = np.array([f[1] for f in flat])
    szero = np.array([f[2] for f in flat])

    wp_h = []
    for h in range(2):
        wp = np.ascontiguousarray(w5t[h][:, si, sj, :])      # [128, S, 128]
        if szero.any():
            wp[:, szero, :] = 0.0
        wp_h.append(wp.reshape(128, S * 128))

    bias3 = bias.reshape(NUM_NODES, 2, 128)
    bias_h = [np.ascontiguousarray(bias3[:, h, :].T) for h in range(2)]

    # fp8 DoubleRow operands: W*16 / x/16 so fp8 products share the fp16
    # PSUM scale. wq8_h[h][:, plane, pairslot*128+o]; plane0=jA, plane1=jB.
    w5f = weight32.reshape(NUM_NODES, 2, 128, NUM_NODES, IN_F)
    NCOV = sum(len(pc) for pc in cov)
    wq8_h = []
    for h in range(2):
        wq = np.zeros((128, 2, max(NCOV, 1) * 128), dtype=f8np)
        g = 0
        for k, (i, _js, _z) in enumerate(sched):
            for pi, a, b in cov[k]:
                for plane, j in ((0, a), (1, b)):
                    blk = w5f[i, h, :, j, :] * 16.0      # [o=128, kf=128]
                    wq[:, plane, g * 128:(g + 1) * 128] =                         blk.T.astype(f8np)
                g += 1
        wq8_h.append(wq)

    xq8_q = []
    for bq in range(P_BATCH):
        xc = x[bq * B_C:(bq + 1) * B_C] / 16.0           # [1024, 21, 128]
        xq = np.zeros((128, 2, NPAIR * N_PH * B_TILE), dtype=f8np)
        x4 = xc.reshape(N_PH, B_TILE, NUM_NODES, IN_F)   # h, b, j, p
        for pi, (a, b) in enumerate(pairs):
            for plane, j in ((0, a), (1, b)):
                arr = x4[:, :, j, :].transpose(2, 0, 1)  # p, h, b
                xq[:, plane, pi * N_PH * B_TILE:(pi + 1) * N_PH * B_TILE] =                     arr.reshape(128, N_PH * B_TILE).astype(f8np)
        xq8_q.append(xq)

    xt_q = []
    for bq in range(P_BATCH):
        xc = x16[bq * B_C:(bq + 1) * B_C]                    # [1024, 21, 128]
        xc4 = xc.reshape(N_PH, B_TILE, NUM_NODES, IN_F)      # ph, b, j, p
        xr = xc4[:, :, xorder, :]                            # ph, b, s, p
        xt = np.ascontiguousarray(xr.transpose(3, 0, 2, 1))  # p, ph, s, b
        xt_q.append(xt.reshape(128, N_PH * NX * B_TILE))

    in_maps = []
    for c in range(N_CORES):
        bq, h = divmod(c, 2)
        in_maps.append({
            "xt": xt_q[bq],
            "wp": wp_h[h],
            "wq8": wq8_h[h],
            "xq8": xq8_q[bq],
            "biasr": bias_h[h],
        })
    return in_maps


def _gather_output(results):
    y = np.empty((P_BATCH, B_C, NUM_NODES, 2, 128), dtype=np.float32)
    for c in range(N_CORES):
        bq, h = divmod(c, 2)
        oc = results[c]["out"].astype(np.float32).reshape(NUM_NODES, 128, B_C)
        y[bq, :, :, h, :] = oc.transpose(2, 0, 1)
    return y.reshape(BATCH, NUM_NODES, OUT_F)


def _ensure_axon_profile_hook():
    """Provide antenv.axon_hooks if the image lacks it (no-op otherwise).

    concourse.bass_utils imports antenv.axon_hooks on the trace path; some
    images miss the module, which would turn BASS_TRACE=1 into an
    ImportError. Registers the standard ctypes NTFF hook when possible.
    """
    try:
        import antenv.axon_hooks  # noqa: F401
        return
    except ImportError:
        pass
    try:
        import antenv
    except ImportError:
        return
    import contextlib
    import ctypes
    import sys
    import types

    hook = None
    try:
        lib = ctypes.CDLL("/opt/axon/libaxon_pjrt.so")
        if hasattr(lib, "axon_start_nrt_profile"):
            lib.axon_start_nrt_profile.argtypes = [
                ctypes.POINTER(ctypes.c_int64), ctypes.c_size_t]
            lib.axon_start_nrt_profile.restype = ctypes.c_int64
            lib.axon_stop_nrt_profile.argtypes = [ctypes.c_char_p]
            lib.axon_stop_nrt_profile.restype = ctypes.c_int64

            @contextlib.contextmanager
            def hook(output_dir, device_ids):
                import jax
                jax.devices()
                if device_ids:
                    ids = (ctypes.c_int64 * len(device_ids))(*device_ids)
                    rc = lib.axon_start_nrt_profile(ids, len(device_ids))
                else:
                    rc = lib.axon_start_nrt_profile(None, 0)
                if rc != 0:
                    raise RuntimeError(f"axon_start_nrt_profile rc={rc}")
                try:
                    yield
                finally:
                    lib.axon_stop_nrt_profile(str(output_dir).encode())
    except OSError:
        hook = None

    mod = types.ModuleType("antenv.axon_hooks")
    mod._hook = hook
    mod.get_axon_ntff_profile_hook = lambda: mod._hook

    def _set(h):
        mod._hook = h

    mod.set_axon_ntff_profile_hook = _set
    sys.modules["antenv.axon_hooks"] = mod
    antenv.axon_hooks = mod


def kernel(x, weight, bias, adjacency):
    from concourse.bass_utils import run_bass_kernel_spmd

    _ensure_axon_profile_hook()
    nc, sched = _get_graph(adjacency)
    in_maps = _pack_inputs(x, weight, bias, sched)

    kwargs = {}
    if os.environ.get("KERNEL_TRACE"):
        kwargs["trace"] = True
        tcores = os.environ.get("KERNEL_TRACE_CORES")
        if tcores:
            kwargs["trace_cores"] = [int(t) for t in tcores.split(",")]

    res = run_bass_kernel_spmd(nc, in_maps, core_ids=list(range(N_CORES)),
                               **kwargs)
    kernel.last_result = res
    return _gather_output(res.results)


kernel.last_result = None

